# revision 21
# baseline (speedup 1.0000x reference)
"""CWT (continuous wavelet transform, pywt 'morl', 5 scales) as a Bass/Tile
kernel for 8 Trainium2 NeuronCores.

Math: for each scale s with integrated-wavelet filter k (length L), the
reference computes  trim(diff(full_corr(x, k))) * (-sqrt(s)) along T.  That
whole pipeline is a single correlation with the fixed kernel
    G[j] = sqrt(s) * (k[j] - k[j-1]),  j = 0..L  (k[-1] = k[L] = 0)
applied with offset  off = floor((L-2)/2) - (L-1):
    y[t] = sum_j x[t + off + j] * G[j]   (x zero-padded outside [0,T))
i.e. y = A_s @ x with the Toeplitz band matrix A_s[t, u] = G[u - t - off].

Kernel strategy (SPMD over 8 cores): core c owns t_out block I = c//2
(512 rows) and B-half h = c%2 (512 of 1024 batch*channel columns).

Large scales (167, 336: band ≥ T) run dense:
    psum[b, s] += X_chunk[q][:, b*128:+128].T @ wbuf[s][:, 1920-128q : +512]
over all 16 t_in chunks; wbuf is a [128 x 2432] sliding-window Toeplitz band
whose per-core t_out shift is baked into its *data* by the host, keeping one
static instruction stream for all cores.

Small scales (1, 27, 76: narrow band) are banded: only nb_s = 6/8/14 t_in
chunks touch a 512-row t_out block.  The host ships each core a shifted
window `xwin` of those 14 chunks (zero-padded at the edges) plus
core-independent mini band buffers, so the banded chains are static too:
    psum[b, s] += xwin_chunk[d0_s+5+j].T @ miniw[s][:, (nb_s-1-j)*128 : +512]

All matmuls are float32r (full-rate fp32 on the PE, ~1.4e-4 rel err),
N=512, accumulating in PSUM.
"""
import sys
import os

sys.path.insert(0, "/opt/trn_rl_repo")

import numpy as np

# ----------------------------------------------------------------- constants
WIDTHS = [1, 27, 76, 167, 336]
T = 2048
B = 1024  # 16 batch * 64 channels
N_CORES = 8
TOUT_PER_CORE = 512  # t_out rows per core (block I = core//2)
BH = 512  # B columns per core (half h = core%2)
NQ = T // 128  # 16 t_in chunks
NBH = BH // 128  # 4 column blocks per core
WBUF_W = 1920 + TOUT_PER_CORE  # dense band buffer width per large scale
BIG = [3, 4]  # scale indices processed densely
SMALL = [0, 1, 2]  # banded scale indices
XWIN_LO = -5  # xwin covers absolute chunks 4*blk + XWIN_LO .. +8
XWIN_NCH = 14

LAST_EXEC_NS = None  # set when CWT_TRACE=1


def _filters():
    """pywt 'morl' integrated wavelet, resampled per scale (matches reference)."""
    precision = 10
    n = 2**precision
    lb, ub = -8.0, 8.0
    t = np.linspace(lb, ub, n)
    psi = np.exp(-(t**2) / 2.0) * np.cos(5.0 * t)
    step = t[1] - t[0]
    int_psi = np.cumsum(psi) * step
    filts = []
    for scale in WIDTHS:
        j = (np.arange(scale * (ub - lb) + 1) / (scale * step)).astype(np.int64)
        j = j[j < n]
        filts.append(int_psi[j].astype(np.float32))
    return filts


def _g_kernels():
    """Effective correlation kernels G_s (len L+1) and offsets off_s."""
    gs = []
    for s, k in zip(WIDTHS, _filters()):
        k64 = k.astype(np.float64)
        L = len(k64)
        G = (np.sqrt(s) * np.diff(np.concatenate([[0.0], k64, [0.0]]))).astype(
            np.float32
        )
        off = int(np.floor((L - 2) / 2.0)) - (L - 1)
        gs.append((G, off))
    return gs


def _small_params():
    """(scale_idx, d0, nb, C, W) per banded scale."""
    gs = _g_kernels()
    out = []
    for si in SMALL:
        G, off = gs[si]
        L1 = len(G)
        d0 = off // 128
        span_hi = 511 + off + (L1 - 1) - 128 * d0
        nb = -(-(span_hi + 1) // 128)
        C = 128 * d0 + 128 * (nb - 1) - off
        W = 512 + 128 * (nb - 1)
        out.append((si, d0, nb, C, W))
    return out


def _toeplitz(G, C, W):
    p = np.arange(128)[:, None]
    w = np.arange(W)[None, :]
    idx = p - w + C
    valid = (idx >= 0) & (idx < len(G))
    return np.where(valid, G[np.clip(idx, 0, len(G) - 1)], np.float32(0.0)).astype(
        np.float32
    )


def _make_wbufs():
    """Per-t_out-block dense band buffers for the large scales."""
    gs = _g_kernels()
    bufs = []
    for blk in range(T // TOUT_PER_CORE):
        rc = blk * TOUT_PER_CORE
        per_scale = []
        for si in BIG:
            G, off = gs[si]
            per_scale.append(_toeplitz(G, 1920 - rc - off, WBUF_W))
        buf = np.stack(per_scale, axis=1).reshape(128, len(BIG) * WBUF_W)
        bufs.append(np.ascontiguousarray(buf))
    return bufs


def _make_miniw():
    """Core-independent banded buffers for the small scales, concatenated."""
    gs = _g_kernels()
    parts = [_toeplitz(gs[si][0], C, W) for si, d0, nb, C, W in _small_params()]
    return np.ascontiguousarray(np.concatenate(parts, axis=1))


_CONST_CACHE = None


def _consts():
    global _CONST_CACHE
    if _CONST_CACHE is None:
        _CONST_CACHE = (_make_wbufs(), _make_miniw(), _small_params())
    return _CONST_CACHE


# ----------------------------------------------------------------- program
_NC_CACHE = None


def _build_program():
    import concourse.bass as bass
    import concourse.bacc as bacc
    import concourse.mybir as mybir
    import concourse.tile as tile

    sp = _small_params()
    mw_w = sum(W for _, _, _, _, W in sp)

    nc = bacc.Bacc(None, target_bir_lowering=False, debug=False)

    x_d = nc.declare_dram_parameter("x", [T, BH], mybir.dt.float32r, isOutput=False)
    xw_d = nc.declare_dram_parameter(
        "xwin", [XWIN_NCH * 128, BH], mybir.dt.float32r, isOutput=False
    )
    w_d = nc.declare_dram_parameter(
        "wbuf", [128, len(BIG) * WBUF_W], mybir.dt.float32r, isOutput=False
    )
    mw_d = nc.declare_dram_parameter(
        "miniw", [128, mw_w], mybir.dt.float32r, isOutput=False
    )
    out_d = nc.declare_dram_parameter(
        "out", [5, 128, NBH * TOUT_PER_CORE], mybir.dt.float32, isOutput=True
    )

    with tile.TileContext(nc) as tc:
        with (
            tc.tile_pool(name="xp", bufs=1) as xp,
            tc.tile_pool(name="wp", bufs=1) as wp,
            tc.tile_pool(name="op", bufs=1) as op,
            tc.tile_pool(name="pp", bufs=1, space=bass.MemorySpace.PSUM) as pp,
        ):
            # band buffers ride the ACT HWDGE ring so the x/xwin stream on
            # the SP ring doesn't queue behind them
            # ~0.65us issue per dma_start and ~165GB/s per queue: keep
            # pieces around 0.5-1MB so transfers spread across queues
            wsb = wp.tile([128, len(BIG) * WBUF_W], mybir.dt.float32r, tag="wsb")
            wh = WBUF_W // 2
            for i in range(len(BIG)):
                for lo, hi in ((0, wh), (wh, WBUF_W)):
                    nc.scalar.dma_start(
                        wsb[:, i * WBUF_W + lo : i * WBUF_W + hi],
                        w_d[:, i * WBUF_W + lo : i * WBUF_W + hi],
                    )
            mwsb = wp.tile([128, mw_w], mybir.dt.float32r, tag="mwsb")
            mh = mw_w // 2
            nc.scalar.dma_start(mwsb[:, :mh], mw_d[:, :mh])
            nc.scalar.dma_start(mwsb[:, mh:], mw_d[:, mh:])

            # x chunks: grouped DMAs (each dma_start costs ~650ns of HWDGE
            # issue; Tile dependency tracking is AP-range based so consumers
            # wait only for their group)
            xsb = xp.tile([128, NQ * BH], mybir.dt.float32r, tag="xsb", name="xsb")
            for g0, g1 in ((0, 1), (1, 2), (2, 4), (4, 6), (6, 8), (8, 10), (10, 12), (12, 14), (14, NQ)):
                nc.sync.dma_start(
                    xsb[:, g0 * BH : g1 * BH].rearrange("p (q b) -> p q b", b=BH),
                    x_d[g0 * 128 : g1 * 128, :].rearrange("(q p) b -> p q b", p=128),
                )
            xwsb = xp.tile(
                [128, XWIN_NCH * BH], mybir.dt.float32r, tag="xwsb", name="xwsb"
            )
            for g0, g1 in ((0, 4), (4, 7), (7, 11), (11, XWIN_NCH)):
                nc.sync.dma_start(
                    xwsb[:, g0 * BH : g1 * BH].rearrange("p (q b) -> p q b", b=BH),
                    xw_d[g0 * 128 : g1 * 128, :].rearrange("(q p) b -> p q b", p=128),
                )

            grp = 0

            def run_chain(s, ps, mk_ops):
                # q-major interleave across the 4 b-chains: one arriving
                # x-chunk feeds 4 matmuls, keeping the PE arrival-paced
                # instead of stalling on a single chain's last chunk
                nonlocal grp
                stg = stgs[s]
                psums = []
                opl = []
                for b in range(NBH):
                    psums.append(
                        pp.tile(
                            [128, TOUT_PER_CORE],
                            mybir.dt.float32,
                            tag=f"ps{(grp % 2) * 4 + b}",
                            name=f"ps_{s}_{b}",
                        )
                    )
                    opl.append(mk_ops(b))
                nq = len(opl[0])
                for i in range(nq):
                    for b in range(NBH):
                        lhsT, rhs = opl[b][i]
                        nc.tensor.matmul(
                            psums[b][:],
                            lhsT,
                            rhs,
                            start=(i == 0),
                            stop=(i == nq - 1),
                        )
                for b in range(NBH):
                    nc.vector.tensor_copy(
                        stg[:, b * TOUT_PER_CORE : (b + 1) * TOUT_PER_CORE],
                        psums[b][:],
                    )
                grp += 1

            stgs = {}
            for s in BIG + SMALL:
                stgs[s] = op.tile(
                    [128, NBH * TOUT_PER_CORE],
                    mybir.dt.float32,
                    tag=f"stg{s}",
                    name=f"stg{s}",
                )

            # dense large scales first (need only x + their wbuf region)
            for i, s in enumerate(BIG):
                def mk_dense(b, i=i):
                    return [
                        (
                            xsb[:, q * BH + b * 128 : q * BH + (b + 1) * 128],
                            wsb[
                                :,
                                i * WBUF_W
                                + 1920
                                - 128 * q : i * WBUF_W
                                + 1920
                                - 128 * q
                                + TOUT_PER_CORE,
                            ],
                        )
                        for q in range(NQ)
                    ]

                run_chain(s, None, mk_dense)
                nc.scalar.dma_start(out_d[s], stgs[s][:])

            # banded small scales (need xwin + miniw)
            mw_base = 0
            for si, d0, nb, C, W in sp:
                def mk_small(b, d0=d0, nb=nb, base=mw_base):
                    ops = []
                    for j in range(nb):
                        cj = d0 - XWIN_LO + j
                        ops.append(
                            (
                                xwsb[:, cj * BH + b * 128 : cj * BH + (b + 1) * 128],
                                mwsb[
                                    :,
                                    base
                                    + (nb - 1 - j) * 128 : base
                                    + (nb - 1 - j) * 128
                                    + TOUT_PER_CORE,
                                ],
                            )
                        )
                    return ops

                run_chain(si, None, mk_small)
                if si == SMALL[-1]:
                    half = NBH * TOUT_PER_CORE // 2
                    nc.scalar.dma_start(out_d[si][:, :half], stgs[si][:, :half])
                    nc.scalar.dma_start(out_d[si][:, half:], stgs[si][:, half:])
                else:
                    nc.scalar.dma_start(out_d[si], stgs[si][:])
                mw_base += W

    nc.compile()  # bacc passes legalize multi-sem-waits for walrus codegen
    return nc


def _program():
    global _NC_CACHE
    if _NC_CACHE is None:
        _NC_CACHE = _build_program()
    return _NC_CACHE


# ----------------------------------------------------------------- entry
def kernel(x: np.ndarray) -> np.ndarray:
    """x: [16, 2048, 64] float32 -> [16, 2048, 64, 5] float32"""
    global LAST_EXEC_NS
    from concourse.bass_utils import run_bass_kernel_spmd

    n, t, c = x.shape
    assert (t, n * c) == (T, B), (x.shape,)

    X = np.ascontiguousarray(x.transpose(1, 0, 2).reshape(T, B).astype(np.float32))
    wbufs, miniw, sp = _consts()
    in_maps = []
    for core in range(N_CORES):
        blk, h = core // 2, core % 2
        xh = X[:, h * BH : (h + 1) * BH]
        lo = (4 * blk + XWIN_LO) * 128
        xwin = np.zeros((XWIN_NCH * 128, BH), np.float32)
        a = max(0, lo)
        bnd = min(T, lo + XWIN_NCH * 128)
        if bnd > a:
            xwin[a - lo : bnd - lo, :] = xh[a:bnd, :]
        in_maps.append(
            {
                "x": np.ascontiguousarray(xh),
                "xwin": xwin,
                "wbuf": wbufs[blk],
                "miniw": miniw,
            }
        )

    nc = _program()
    trace = bool(int(os.environ.get("CWT_TRACE", "0")))
    res = run_bass_kernel_spmd(nc, in_maps, list(range(N_CORES)), trace=trace)
    if trace:
        LAST_EXEC_NS = res.exec_time_ns
        globals()["LAST_RESULTS"] = res

    # per-core out: [5, 128, NBH*512] -> assemble [5, T, B]
    Y = np.empty((5, T, B), np.float32)
    for core in range(N_CORES):
        blk, h = core // 2, core % 2
        o = res.results[core]["out"].reshape(5, 128, NBH, TOUT_PER_CORE)
        # [s, p, b, n] -> Y[s, 512*blk + n, h*512 + 128b + p]
        Y[
            :, blk * TOUT_PER_CORE : (blk + 1) * TOUT_PER_CORE,
            h * BH : (h + 1) * BH,
        ] = o.transpose(0, 3, 2, 1).reshape(5, TOUT_PER_CORE, BH)
    return np.ascontiguousarray(
        Y.reshape(5, T, n, c).transpose(2, 1, 3, 0).astype(np.float32)
    )
